# revision 16
# baseline (speedup 1.0000x reference)
"""Trainium2 Bass kernel for nn_ComplexCrossAttention.

Sharding: 8 cores = 2 batches x 4 head-groups (4 heads each).

Key structure (v2):
  - Activations arrive pre-transposed and pre-cast to bf16 on host:
    xt/ct are [2, C, L] = (xr^T, xi^T); the Karatsuba sum stream
    (xr+xi)^T is computed on-chip (1 DVE add per ck chunk).
  - All four complex projections (Q, K, V, O) use the Karatsuba
    3-multiplication complex product:
        t1 = ar@br, t2 = ai@bi, t3 = (ar+ai)@(br+bi)
        re = t1 - t2,  im = t3 - t1 - t2
    which cuts projection PE time by 25% vs the 4-mult stacked form.
  - scores^T = (qr.kr + qi.ki)*scale with s on partitions (single
    stacked d2=128 contraction - already minimal at 2 real products).
  - softmax: exp on Act engine, in-place pairwise tree-sum of the 16
    s-tiles on Pool engine, cross-partition denominator via one
    ones-matmul, reciprocal+normalize on DVE.
  - O-projection in two PSUM waves (t1,t2 then t3) so the whole
    attention inner loop fits in exactly 8 PSUM banks; O-chunks of
    l-block lb-1 are emitted interleaved with the heads of lb so the
    PE never drains while Act catches up on exp.
Host sums the 4 partial y per batch and adds the bias.
"""

import sys

import numpy as np

try:
    import concourse.bacc as bacc
except ImportError:  # pragma: no cover - fallback for bare environments
    sys.path.insert(0, "/opt/trn_rl_repo")
    import concourse.bacc as bacc

import concourse.mybir as mybir
import concourse.tile as tile
from concourse.bass_utils import run_bass_kernel_spmd

F32 = mybir.dt.float32
F32R = mybir.dt.float32r
BF16 = mybir.dt.bfloat16

# ---- problem constants (hardcoded per contract) ----
B, L, S, C = 2, 2048, 2048, 1024
H, D = 16, 64
SCALE = float(1.0 / np.sqrt(np.float32(D)))
HPC = 4          # heads per core
NHP = 2          # head pairs per core
D2 = 2 * D       # stacked (real|imag) head dim = 128
NCK = C // 128   # contraction chunks = 8
NLB = L // 512   # l-blocks = 4
NST = S // 128   # s-tiles = 16
NEB = 2          # e-blocks of 512 in C

QS_DT = F32R     # scores operands
EXP_DT = BF16    # expT / vs / ones
OT_DT = BF16     # ot / wo

_CACHE = {}


DEBUG_DUMPS = False


def _build_program():
    nc = bacc.Bacc("TRN2", target_bir_lowering=False, debug=False, num_devices=8)

    # per-core external inputs (host-prepared layouts)
    xt = nc.dram_tensor("xt", [2, C, L], BF16, kind="ExternalInput")
    ct = nc.dram_tensor("ct", [2, C, S], BF16, kind="ExternalInput")
    # wq/wk: [C, NHP, 3, 128]; product streams t=0:wr 1:wi 2:wr+wi,
    # head pair packed along the last (M) axis.
    wq = nc.dram_tensor("wq", [C, NHP, 3, D2], BF16, kind="ExternalInput")
    wk = nc.dram_tensor("wk", [C, NHP, 3, D2], BF16, kind="ExternalInput")
    # wv: [C, 3, HPC*64] rhs product streams
    wv = nc.dram_tensor("wv", [C, 3, HPC * D], BF16, kind="ExternalInput")
    # wo: [NHP, 128, 3, NEB, 512]; rows = head-pair packed d
    wo = nc.dram_tensor("wo", [NHP, D2, 3, NEB, 512], BF16, kind="ExternalInput")

    y_r = nc.dram_tensor("y_r", [L, C], F32, kind="ExternalOutput")
    y_i = nc.dram_tensor("y_i", [L, C], F32, kind="ExternalOutput")
    dbg = None
    if DEBUG_DUMPS:
        dbg = {
            "vs_d": nc.dram_tensor("vs_d", [128, NST, HPC * D2], BF16, kind="ExternalOutput"),
            "ot_d": nc.dram_tensor("ot_d", [NLB, 128, 3, NHP, 512], BF16, kind="ExternalOutput"),
            "den_d": nc.dram_tensor("den_d", [NLB, HPC, 128, 512], F32, kind="ExternalOutput"),
        }

    with tile.TileContext(nc) as tc:
        _emit(nc, tc, xt, ct, wq, wk, wv, wo, y_r, y_i, dbg)

    nc.compile()
    return nc


def _proj_karatsuba(nc, at_sb, w_sb, out_sb, ps_pool, nblk):
    """Q/K-style projection: out_sb[d2, h, n] for one activation stream.

    at_sb: [128, NCK, 3, N] bf16 transposed activation product streams
    w_sb:  [128, NCK, NHP, 3, 128] bf16 lhsT product streams
    out_sb: [128, HPC, N] f32r, partitions = (re | im) per head
    """
    for hp in range(NHP):
        for nb in range(nblk):
            nsl = slice(nb * 512, (nb + 1) * 512)
            pt = ps_pool.tile([128, 3, 512], F32, tag="pt", name="pt")
            for ck in range(NCK):
                for p in range(3):
                    nc.tensor.matmul(
                        pt[:, p, :],
                        w_sb[:, ck, hp, p, :],
                        at_sb[:, ck, p, nsl],
                        start=(ck == 0),
                        stop=(ck == NCK - 1),
                    )
            for j in range(2):
                h = 2 * hp + j
                r = slice(64 * j, 64 * (j + 1))
                # re = t1 - t2 ; im = t3 - t1 - t2
                nc.vector.tensor_sub(out=out_sb[0:64, h, nsl], in0=pt[r, 0, :], in1=pt[r, 1, :])
                nc.vector.tensor_sub(out=out_sb[64:128, h, nsl], in0=pt[r, 2, :], in1=pt[r, 0, :])
                nc.vector.tensor_sub(
                    out=out_sb[64:128, h, nsl], in0=out_sb[64:128, h, nsl], in1=pt[r, 1, :]
                )


def _load_streams(nc, dst_sb, src_dram, n):
    """Load transposed activation streams per ck and form the sum stream."""
    for ck in range(NCK):
        csl = slice(ck * 128, (ck + 1) * 128)
        for p in range(2):
            nc.sync.dma_start(out=dst_sb[:, ck, p, :], in_=src_dram[p, csl, :])
        nc.vector.tensor_add(
            out=dst_sb[:, ck, 2, :], in0=dst_sb[:, ck, 0, :], in1=dst_sb[:, ck, 1, :]
        )


def _emit(nc, tc, xt, ct, wq, wk, wv, wo, y_r, y_i, dbg=None):
    from contextlib import ExitStack

    ctx = ExitStack()
    with ctx:
        attn_sb = ctx.enter_context(tc.tile_pool(name="attn_sb", bufs=1))

        # persistent attention operands
        qs = attn_sb.tile([128, HPC, L], QS_DT)            # [(qr|qi), h, l]
        ks = attn_sb.tile([128, HPC, S], QS_DT)            # [(kr|ki), h, s]
        vs = attn_sb.tile([128, NST, HPC * D2], EXP_DT)    # [s-part, st, (vr|vi) per head]

        # ---------- P1: Q projection ----------
        with (
            tc.tile_pool(name="xtp", bufs=1) as xt_pool,
            tc.tile_pool(name="wqp", bufs=1) as wq_pool,
            tc.tile_pool(name="psq", bufs=2, space="PSUM") as ps_q,
        ):
            wq_sb = wq_pool.tile([128, NCK, NHP, 3, D2], BF16, name="wq_sb")
            wq_r = wq.rearrange("(ck p) hp t m -> p ck hp t m", p=128)
            for ck in range(NCK):
                nc.sync.dma_start(out=wq_sb[:, ck], in_=wq_r[:, ck])
            xt_sb = xt_pool.tile([128, NCK, 3, L], BF16, name="xt_sb")
            _load_streams(nc, xt_sb, xt, L)
            _proj_karatsuba(nc, xt_sb, wq_sb, qs, ps_q, NLB)

        # ---------- P2+P3: K and V projections ----------
        with (
            tc.tile_pool(name="ctp", bufs=1) as ct_pool,
            tc.tile_pool(name="wkp", bufs=1) as wk_pool,
        ):
            wk_sb = wk_pool.tile([128, NCK, NHP, 3, D2], BF16, name="wk_sb")
            wk_r = wk.rearrange("(ck p) hp t m -> p ck hp t m", p=128)
            for ck in range(NCK):
                nc.sync.dma_start(out=wk_sb[:, ck], in_=wk_r[:, ck])
            wv_sb = wk_pool.tile([128, NCK, 3, HPC * D], BF16, name="wv_sb")
            wv_r = wv.rearrange("(ck p) t n -> p ck t n", p=128)
            nc.sync.dma_start(out=wv_sb, in_=wv_r)
            ct_sb = ct_pool.tile([128, NCK, 3, S], BF16, name="ct_sb")
            _load_streams(nc, ct_sb, ct, S)
            with tc.tile_pool(name="psk", bufs=2, space="PSUM") as ps_k:
                _proj_karatsuba(nc, ct_sb, wk_sb, ks, ps_k, NLB)

            # V: out vs[s-part, st, d2all]; products along free dim
            with tc.tile_pool(name="psv", bufs=2, space="PSUM") as ps_v:
                for st in range(NST):
                    ssl = slice(st * 128, (st + 1) * 128)
                    pv = ps_v.tile([128, 3, 512], F32, tag="pv", name="pv")
                    for ck in range(NCK):
                        for p in range(3):
                            nc.tensor.matmul(
                                pv[:, p, 0:HPC * D],
                                ct_sb[:, ck, p, ssl],
                                wv_sb[:, ck, p, :],
                                start=(ck == 0),
                                stop=(ck == NCK - 1),
                                skip_group_check=True,
                            )
                    vw = vs[:, st, :].rearrange("p (h x) -> p h x", h=HPC)
                    p0 = pv[:, 0, 0:HPC * D].rearrange("p (h x) -> p h x", h=HPC)
                    p1 = pv[:, 1, 0:HPC * D].rearrange("p (h x) -> p h x", h=HPC)
                    p2 = pv[:, 2, 0:HPC * D].rearrange("p (h x) -> p h x", h=HPC)
                    nc.vector.tensor_sub(out=vw[:, :, 0:64], in0=p0, in1=p1)
                    nc.vector.tensor_sub(out=vw[:, :, 64:128], in0=p2, in1=p0)
                    nc.vector.tensor_sub(out=vw[:, :, 64:128], in0=vw[:, :, 64:128], in1=p1)
                if dbg is not None:
                    nc.sync.dma_start(out=dbg["vs_d"][:, :, :], in_=vs)

        # ---------- P4: attention + output projection ----------
        with (
            tc.tile_pool(name="late", bufs=1) as late_pool,
            tc.tile_pool(name="expp", bufs=2) as exp_pool,
            tc.tile_pool(name="otp", bufs=2) as ot_pool,
            tc.tile_pool(name="rcp", bufs=2) as rc_pool,
            tc.tile_pool(name="ysb", bufs=3) as ysb_pool,
            tc.tile_pool(name="pss", bufs=2, space="PSUM") as ps_s,
            tc.tile_pool(name="pso", bufs=1, space="PSUM") as ps_o,
            tc.tile_pool(name="psd", bufs=1, space="PSUM") as ps_d,
            tc.tile_pool(name="psy", bufs=1, space="PSUM") as ps_y,
        ):
            wo_sb = late_pool.tile([128, NHP, 3, NEB, 512], OT_DT, name="wo_sb")
            ones = late_pool.tile([128, D2], EXP_DT, name="ones")
            nc.vector.memset(ones, 1.0)
            nc.sync.dma_start(out=wo_sb, in_=wo.rearrange("hp p t eb e -> p hp t eb e"))
            ot_tiles = {}

            def emit_o_chunk(lb, half):
                """Output projection for one (jt, eb) pair-range of l-block lb.

                Two PSUM waves: wave1 = t1,t2 (4 matmuls into 2 banks),
                wave2 = t3 (2 matmuls into the recycled tile).
                """
                ot3 = ot_tiles[lb]
                for jt_eb in range(2 * half, 2 * half + 2):
                    jt, eb = jt_eb // 2, jt_eb % 2
                    lrow = slice((lb * 4 + jt) * 128, (lb * 4 + jt + 1) * 128)
                    jsl = slice(jt * 128, (jt + 1) * 128)
                    esl = slice(eb * 512, (eb + 1) * 512)
                    ty = ps_y.tile([128, 2, 512], F32, tag="ty", name="ty")
                    for p in range(2):
                        for hp in range(NHP):
                            nc.tensor.matmul(
                                ty[:, p, :],
                                ot3[:, p, hp, jsl],
                                wo_sb[:, hp, p, eb, :],
                                start=(hp == 0),
                                stop=(hp == NHP - 1),
                            )
                    yr_t = ysb_pool.tile([128, 512], F32, tag="yr", name="yr_t")
                    nc.vector.tensor_sub(out=yr_t, in0=ty[:, 0, :], in1=ty[:, 1, :])
                    nc.sync.dma_start(out=y_r[lrow, esl], in_=yr_t)
                    u_t = ysb_pool.tile([128, 512], F32, tag="u", name="u_t")
                    nc.vector.tensor_add(out=u_t, in0=ty[:, 0, :], in1=ty[:, 1, :])
                    ty2 = ps_y.tile([128, 2, 512], F32, tag="ty", name="ty2")
                    for hp in range(NHP):
                        nc.tensor.matmul(
                            ty2[:, 0, :],
                            ot3[:, 2, hp, jsl],
                            wo_sb[:, hp, 2, eb, :],
                            start=(hp == 0),
                            stop=(hp == NHP - 1),
                        )
                    yi_t = ysb_pool.tile([128, 512], F32, tag="yi", name="yi_t")
                    nc.vector.tensor_sub(out=yi_t, in0=ty2[:, 0, :], in1=u_t)
                    nc.sync.dma_start(out=y_i[lrow, esl], in_=yi_t)

            for lb in range(NLB):
                lsl = slice(lb * 512, (lb + 1) * 512)
                ot3 = ot_pool.tile([128, 3, NHP, 512], OT_DT, tag="ot", name="ot3")
                ot_tiles[lb] = ot3
                for h in range(HPC):
                    expt = exp_pool.tile([128, NST, 512], EXP_DT, tag="expt", name="expt")
                    for pr in range(NST // 2):
                        pscore = ps_s.tile([128, 2, 512], F32, tag="ps", name="pscore")
                        for j in range(2):
                            st = 2 * pr + j
                            nc.tensor.matmul(
                                pscore[:, j, :],
                                ks[:, h, st * 128:(st + 1) * 128],
                                qs[:, h, lsl],
                                start=True,
                                stop=True,
                                skip_group_check=True,
                            )
                        nc.scalar.activation(
                            out=expt[:, 2 * pr:2 * pr + 2, :],
                            in_=pscore,
                            func=mybir.ActivationFunctionType.Exp,
                            scale=SCALE,
                        )
                    # O-chunk filler from previous l-block keeps PE busy
                    if lb > 0 and h < 2:
                        emit_o_chunk(lb - 1, h)
                    # av: [d2, l] accumulated over s-tiles (reads expt first)
                    pav = ps_o.tile([128, 512], F32, tag="pav", name="pav")
                    for st in range(NST):
                        nc.tensor.matmul(
                            pav,
                            vs[:, st, h * D2:(h + 1) * D2],
                            expt[:, st, :],
                            start=(st == 0),
                            stop=(st == NST - 1),
                        )
                    if lb > 0 and h >= 2:
                        emit_o_chunk(lb - 1, h)
                    # in-place pairwise tree-sum of the 16 s-tiles (Pool engine)
                    for step in (1, 2, 4, 8):
                        for j in range(0, NST, 2 * step):
                            nc.gpsimd.tensor_add(
                                out=expt[:, j, :], in0=expt[:, j, :], in1=expt[:, j + step, :]
                            )
                    pden = ps_d.tile([128, 512], F32, tag="pden", name="pden")
                    nc.tensor.matmul(pden, ones, expt[:, 0, :], start=True, stop=True)
                    recip = rc_pool.tile([128, 512], F32, tag="recip", name="recip")
                    nc.vector.reciprocal(out=recip, in_=pden)
                    if dbg is not None:
                        nc.sync.dma_start(out=dbg["den_d"][lb, h], in_=recip)
                    hp, jj = h // 2, h % 2
                    r = slice(64 * jj, 64 * (jj + 1))
                    nc.vector.tensor_mul(out=ot3[r, 0, hp, :], in0=pav[0:64, :], in1=recip[0:64, :])
                    nc.vector.tensor_mul(
                        out=ot3[r, 1, hp, :], in0=pav[64:128, :], in1=recip[64:128, :]
                    )
                # sum stream for the O Karatsuba product
                nc.vector.tensor_add(
                    out=ot3[:, 2, :, :], in0=ot3[:, 0, :, :], in1=ot3[:, 1, :, :]
                )
                if dbg is not None:
                    nc.sync.dma_start(out=dbg["ot_d"][lb], in_=ot3)
            for half in range(4):
                emit_o_chunk(NLB - 1, half)


def _prep_shared(inputs):
    """Batch-shared transposed bf16 activation streams."""
    import ml_dtypes

    bf = ml_dtypes.bfloat16
    out = {}
    for b in range(B):
        xr = inputs["inputs_real"][b].T
        xi = inputs["inputs_imag"][b].T
        cr = inputs["context_real"][b].T
        ci = inputs["context_imag"][b].T
        out[b] = (
            np.ascontiguousarray(np.stack([xr, xi]).astype(bf)),
            np.ascontiguousarray(np.stack([cr, ci]).astype(bf)),
        )
    return out


def _prep_core_inputs(inputs, core, shared=None):
    """Slice + host-prepare the weight layouts for one core."""
    import ml_dtypes

    bf = ml_dtypes.bfloat16
    b = core // 4
    g = core % 4
    hcols = slice(g * HPC * D, (g + 1) * HPC * D)  # 256 channel cols/rows

    if shared is None:
        shared = _prep_shared(inputs)
    xt3, ct3 = shared[b]

    def stack_lhst(wr, wi):
        # [C, NHP, 3, 128]: head-pair packed along M
        wrp = wr.reshape(C, NHP, D2)
        wip = wi.reshape(C, NHP, D2)
        return np.stack([wrp, wip, wrp + wip], axis=2).astype(bf)

    def stack_rhs_v(wr, wi):
        # [C, 3, HPC*D]
        return np.stack([wr, wi, wr + wi], axis=1).astype(bf)

    def stack_wo(wr, wi):
        # [NHP, 128, 3, NEB, 512]: rows = head-pair packed d
        wrp = wr.reshape(NHP, D2, C)
        wip = wi.reshape(NHP, D2, C)
        st = np.stack([wrp, wip, wrp + wip], axis=2)  # [NHP, 128, 3, C]
        return np.ascontiguousarray(st.reshape(NHP, D2, 3, NEB, 512)).astype(bf)

    return {
        "xt": xt3,
        "ct": ct3,
        "wq": stack_lhst(inputs["wq_r"][:, hcols], inputs["wq_i"][:, hcols]),
        "wk": stack_lhst(inputs["wk_r"][:, hcols], inputs["wk_i"][:, hcols]),
        "wv": stack_rhs_v(inputs["wv_r"][:, hcols], inputs["wv_i"][:, hcols]),
        "wo": stack_wo(inputs["wo_r"][hcols, :], inputs["wo_i"][hcols, :]),
    }


def get_program():
    if "nc" not in _CACHE:
        _CACHE["nc"] = _build_program()
    return _CACHE["nc"]


def kernel(**inputs):
    nc = get_program()
    shared = _prep_shared(inputs)
    in_maps = [_prep_core_inputs(inputs, core, shared) for core in range(8)]
    res = run_bass_kernel_spmd(nc, in_maps, core_ids=list(range(8)))

    yr = np.zeros((B, L, C), np.float32)
    yi = np.zeros((B, L, C), np.float32)
    for core in range(8):
        b = core // 4
        yr[b] += res.results[core]["y_r"]
        yi[b] += res.results[core]["y_i"]
    yr += inputs["bo_r"][None, None, :]
    yi += inputs["bo_i"][None, None, :]
    return np.stack([yr, yi], axis=0)


# revision 20
# speedup vs baseline: 1.1348x; 1.1348x over previous
"""Trainium2 Bass kernel for nn_ComplexCrossAttention.

Sharding: 8 cores = 2 batches x 4 head-groups (4 heads each).

Structure (v3):
  - Activations arrive pre-transposed, pre-cast to bf16 on host:
    xt/ct = [2, C, L] = (re^T, im^T). The Karatsuba sum stream is
    formed on-chip (DVE, 2-byte fast mode).
  - All four complex projections (Q, K, V, O) use a 3-multiplication
    complex product with only TWO combine ops:
        m1 = (ar+ai) @ br ; m2 = ai @ (br+bi) ; m3 = ar @ (bi-br)
        re = m1 - m2 ; im = m1 + m3
    (weight sums/differences are precomputed on host for free).
    This cuts projection PE time 25% vs the 4-mult stacked form.
  - q/k in fp16 (same PE rate as bf16, 8x finer mantissa -> score
    logits keep baseline precision).
  - scoresT = (qr.kr + qi.ki)*scale with s on partitions; exp on Act;
    in-place pairwise tree-sum of the 16 s-tiles split Pool/DVE;
    cross-partition denominator via one ones-matmul; normalize on DVE.
  - O-projection in two PSUM waves (m1,m2 then m3, with m1 stashed to
    SBUF) so the attention inner loop fits exactly 8 PSUM banks;
    O-chunks of l-block lb-1 are interleaved between the heads of lb
    to keep PE fed while Act catches up on exp.
Host sums the 4 partial y per batch and adds the bias.
"""

import sys

import numpy as np

try:
    import concourse.bacc as bacc
except ImportError:  # pragma: no cover - fallback for bare environments
    sys.path.insert(0, "/opt/trn_rl_repo")
    import concourse.bacc as bacc

import concourse.mybir as mybir
import concourse.tile as tile
from concourse.bass_utils import run_bass_kernel_spmd

F32 = mybir.dt.float32
F16 = mybir.dt.float16
BF16 = mybir.dt.bfloat16

# ---- problem constants (hardcoded per contract) ----
B, L, S, C = 2, 2048, 2048, 1024
H, D = 16, 64
SCALE = float(1.0 / np.sqrt(np.float32(D)))
HPC = 4          # heads per core
NHP = 2          # head pairs per core
D2 = 2 * D       # stacked (re|im) head dim = 128
NCK = C // 128   # contraction chunks = 8
NLB = L // 512   # l-blocks = 4
NST = S // 128   # s-tiles = 16
NEB = 2          # e-blocks of 512 in C

QS_DT = F16      # scores operands
EXP_DT = BF16    # expT / vs / ones
OT_DT = BF16     # ot / wo

_CACHE = {}


def _build_program():
    nc = bacc.Bacc("TRN2", target_bir_lowering=False, debug=False, num_devices=8)

    xt = nc.dram_tensor("xt", [2, C, L], BF16, kind="ExternalInput")
    ct = nc.dram_tensor("ct", [2, C, S], BF16, kind="ExternalInput")
    # wq/wk: [C, NHP, 3, 128]; product streams t=0: br, 1: br+bi, 2: bi-br;
    # head pair packed along the last (M) axis.
    wq = nc.dram_tensor("wq", [C, NHP, 3, D2], BF16, kind="ExternalInput")
    wk = nc.dram_tensor("wk", [C, NHP, 3, D2], BF16, kind="ExternalInput")
    wv = nc.dram_tensor("wv", [C, 3, HPC * D], BF16, kind="ExternalInput")
    wo = nc.dram_tensor("wo", [NHP, D2, 3, NEB, 512], BF16, kind="ExternalInput")

    y_r = nc.dram_tensor("y_r", [L, C], F32, kind="ExternalOutput")
    y_i = nc.dram_tensor("y_i", [L, C], F32, kind="ExternalOutput")

    with tile.TileContext(nc) as tc:
        _emit(nc, tc, xt, ct, wq, wk, wv, wo, y_r, y_i)

    nc.compile()
    return nc


def _emit(nc, tc, xt, ct, wq, wk, wv, wo, y_r, y_i):
    from contextlib import ExitStack

    ctx = ExitStack()
    with ctx:
        attn_sb = ctx.enter_context(tc.tile_pool(name="attn_sb", bufs=1))

        qs = attn_sb.tile([128, HPC, L], QS_DT)            # [(qr|qi), h, l]
        ks = attn_sb.tile([128, HPC, S], QS_DT)            # [(kr|ki), h, s]
        vs = attn_sb.tile([128, NST, HPC * D2], EXP_DT)    # [s, st, (vr|vi) per head]

        proj_ctx = ExitStack()
        ct_pool = proj_ctx.enter_context(tc.tile_pool(name="ctp", bufs=1))
        ct_sb = ct_pool.tile([128, NCK, 2, S], BF16, name="ct_sb")

        # ---------- P1: Q projection ----------
        with (
            tc.tile_pool(name="xtp", bufs=1) as xt_pool,
            tc.tile_pool(name="wqp", bufs=1) as wq_pool,
            tc.tile_pool(name="xsp", bufs=2) as xs_pool,
            tc.tile_pool(name="psq", bufs=2, space="PSUM") as ps_q,
        ):
            wq_sb = wq_pool.tile([128, NCK, NHP, 3, D2], BF16, name="wq_sb")
            wq_r = wq.rearrange("(ck p) hp t m -> p ck hp t m", p=128)
            xt_sb = xt_pool.tile([128, NCK, 2, L], BF16, name="xt_sb")
            # interleave weight + activation loads so chunk ck is complete
            # (weights AND both streams) as early as possible
            for ck in range(NCK):
                csl = slice(ck * 128, (ck + 1) * 128)
                nc.sync.dma_start(out=wq_sb[:, ck], in_=wq_r[:, ck])
                for p in range(2):
                    nc.sync.dma_start(out=xt_sb[:, ck, p, :], in_=xt[p, csl, :])
            # context loads queue right behind (consumed by K/V)
            for ck in range(NCK):
                csl = slice(ck * 128, (ck + 1) * 128)
                for p in range(2):
                    nc.sync.dma_start(out=ct_sb[:, ck, p, :], in_=ct[p, csl, :])

            for lb in range(NLB):
                lsl = slice(lb * 512, (lb + 1) * 512)
                xstage = xs_pool.tile([128, NCK, 512], BF16, tag="xs", name="xstage")
                for ck in range(NCK):
                    nc.vector.tensor_add(
                        out=xstage[:, ck, :], in0=xt_sb[:, ck, 0, lsl], in1=xt_sb[:, ck, 1, lsl]
                    )
                for hp in range(NHP):
                    pt = ps_q.tile([128, 3, 512], F32, tag="pt", name="pt")
                    for ck in range(NCK):
                        rhs = (xstage[:, ck, :], xt_sb[:, ck, 1, lsl], xt_sb[:, ck, 0, lsl])
                        for p in range(3):
                            nc.tensor.matmul(
                                pt[:, p, :],
                                wq_sb[:, ck, hp, p, :],
                                rhs[p],
                                start=(ck == 0),
                                stop=(ck == NCK - 1),
                            )
                    for j in range(2):
                        h = 2 * hp + j
                        r = slice(64 * j, 64 * (j + 1))
                        nc.vector.tensor_sub(out=qs[0:64, h, lsl], in0=pt[r, 0, :], in1=pt[r, 1, :])
                        nc.gpsimd.tensor_add(
                            out=qs[64:128, h, lsl], in0=pt[r, 0, :], in1=pt[r, 2, :]
                        )

        # ---------- P2: K projection ----------
        wk_pool = proj_ctx.enter_context(tc.tile_pool(name="wkp", bufs=1))
        wk_sb = wk_pool.tile([128, NCK, NHP, 3, D2], BF16, name="wk_sb")
        wk_r = wk.rearrange("(ck p) hp t m -> p ck hp t m", p=128)
        for ck in range(NCK):
            nc.sync.dma_start(out=wk_sb[:, ck], in_=wk_r[:, ck])
        wv_sb = wk_pool.tile([128, NCK, 3, HPC * D], BF16, name="wv_sb")
        nc.sync.dma_start(out=wv_sb, in_=wv.rearrange("(ck p) t n -> p ck t n", p=128))
        cs_pool = proj_ctx.enter_context(tc.tile_pool(name="csp", bufs=1))
        cs_sb = cs_pool.tile([128, NCK, S], BF16, name="cs_sb")
        for ck in range(NCK):
            nc.vector.tensor_add(
                out=cs_sb[:, ck, :], in0=ct_sb[:, ck, 0, :], in1=ct_sb[:, ck, 1, :]
            )
        with tc.tile_pool(name="psk", bufs=2, space="PSUM") as ps_k:
            for hp in range(NHP):
                for sb in range(NLB):
                    ssl = slice(sb * 512, (sb + 1) * 512)
                    pt = ps_k.tile([128, 3, 512], F32, tag="pt", name="ptk")
                    for ck in range(NCK):
                        rhs = (cs_sb[:, ck, ssl], ct_sb[:, ck, 1, ssl], ct_sb[:, ck, 0, ssl])
                        for p in range(3):
                            nc.tensor.matmul(
                                pt[:, p, :],
                                wk_sb[:, ck, hp, p, :],
                                rhs[p],
                                start=(ck == 0),
                                stop=(ck == NCK - 1),
                            )
                    for j in range(2):
                        h = 2 * hp + j
                        r = slice(64 * j, 64 * (j + 1))
                        nc.vector.tensor_sub(out=ks[0:64, h, ssl], in0=pt[r, 0, :], in1=pt[r, 1, :])
                        nc.gpsimd.tensor_add(
                            out=ks[64:128, h, ssl], in0=pt[r, 0, :], in1=pt[r, 2, :]
                        )

        # ---------- P3: V projection ----------
        with tc.tile_pool(name="psv", bufs=2, space="PSUM") as ps_v:
            NV = HPC * D
            for st in range(NST):
                ssl = slice(st * 128, (st + 1) * 128)
                pv = ps_v.tile([128, 3, 512], F32, tag="pv", name="pv")
                for ck in range(NCK):
                    lhs = (cs_sb[:, ck, ssl], ct_sb[:, ck, 1, ssl], ct_sb[:, ck, 0, ssl])
                    for p in range(3):
                        nc.tensor.matmul(
                            pv[:, p, 0:NV],
                            lhs[p],
                            wv_sb[:, ck, p, :],
                            start=(ck == 0),
                            stop=(ck == NCK - 1),
                            skip_group_check=True,
                        )
                vw = vs[:, st, :].rearrange("p (h x) -> p h x", h=HPC)
                p0 = pv[:, 0, 0:NV].rearrange("p (h x) -> p h x", h=HPC)
                p1 = pv[:, 1, 0:NV].rearrange("p (h x) -> p h x", h=HPC)
                p2 = pv[:, 2, 0:NV].rearrange("p (h x) -> p h x", h=HPC)
                nc.vector.tensor_sub(out=vw[:, :, 0:64], in0=p0, in1=p1)
                nc.gpsimd.tensor_add(out=vw[:, :, 64:128], in0=p0, in1=p2)
        proj_ctx.close()

        # ---------- P4: attention + output projection ----------
        with (
            tc.tile_pool(name="late", bufs=1) as late_pool,
            tc.tile_pool(name="expp", bufs=2) as exp_pool,
            tc.tile_pool(name="otp", bufs=2) as ot_pool,
            tc.tile_pool(name="rcp", bufs=2) as rc_pool,
            tc.tile_pool(name="ysb", bufs=3) as ysb_pool,
            tc.tile_pool(name="pss", bufs=2, space="PSUM") as ps_s,
            tc.tile_pool(name="pso", bufs=1, space="PSUM") as ps_o,
            tc.tile_pool(name="psd", bufs=1, space="PSUM") as ps_d,
            tc.tile_pool(name="psy", bufs=1, space="PSUM") as ps_y,
        ):
            wo_sb = late_pool.tile([128, NHP, 3, NEB, 512], OT_DT, name="wo_sb")
            ones = late_pool.tile([128, D2], EXP_DT, name="ones")
            nc.vector.memset(ones, 1.0)
            nc.sync.dma_start(out=wo_sb, in_=wo.rearrange("hp p t eb e -> p hp t eb e"))
            ot_tiles = {}

            def emit_o_chunk(lb, half):
                """Output projection for two (jt, eb) pairs of l-block lb."""
                ot3 = ot_tiles[lb]
                for jt_eb in range(2 * half, 2 * half + 2):
                    jt, eb = jt_eb // 2, jt_eb % 2
                    lrow = slice((lb * 4 + jt) * 128, (lb * 4 + jt + 1) * 128)
                    jsl = slice(jt * 128, (jt + 1) * 128)
                    esl = slice(eb * 512, (eb + 1) * 512)
                    # wave 1: m1 = os@wor, m2 = oi@(wor+woi)
                    ty = ps_y.tile([128, 2, 512], F32, tag="ty", name="ty")
                    for p, osrc in ((0, 2), (1, 1)):
                        for hp in range(NHP):
                            nc.tensor.matmul(
                                ty[:, p, :],
                                ot3[:, osrc, hp, jsl],
                                wo_sb[:, hp, p, eb, :],
                                start=(hp == 0),
                                stop=(hp == NHP - 1),
                            )
                    yr_t = ysb_pool.tile([128, 512], F32, tag="yr", name="yr_t")
                    nc.vector.tensor_sub(out=yr_t, in0=ty[:, 0, :], in1=ty[:, 1, :])
                    nc.sync.dma_start(out=y_r[lrow, esl], in_=yr_t)
                    u_t = ysb_pool.tile([128, 512], F32, tag="u", name="u_t")
                    nc.gpsimd.tensor_copy(out=u_t, in_=ty[:, 0, :])
                    # wave 2: m3 = or@(woi-wor) into the recycled bank pair
                    ty2 = ps_y.tile([128, 2, 512], F32, tag="ty", name="ty2")
                    for hp in range(NHP):
                        nc.tensor.matmul(
                            ty2[:, 0, :],
                            ot3[:, 0, hp, jsl],
                            wo_sb[:, hp, 2, eb, :],
                            start=(hp == 0),
                            stop=(hp == NHP - 1),
                        )
                    yi_t = ysb_pool.tile([128, 512], F32, tag="yi", name="yi_t")
                    nc.vector.tensor_add(out=yi_t, in0=ty2[:, 0, :], in1=u_t)
                    nc.sync.dma_start(out=y_i[lrow, esl], in_=yi_t)

            for lb in range(NLB):
                lsl = slice(lb * 512, (lb + 1) * 512)
                ot3 = ot_pool.tile([128, 3, NHP, 512], OT_DT, tag="ot", name="ot3")
                ot_tiles[lb] = ot3
                for h in range(HPC):
                    expt = exp_pool.tile([128, NST, 512], EXP_DT, tag="expt", name="expt")
                    for pr in range(NST // 2):
                        pscore = ps_s.tile([128, 2, 512], F32, tag="ps", name="pscore")
                        for j in range(2):
                            st = 2 * pr + j
                            nc.tensor.matmul(
                                pscore[:, j, :],
                                ks[:, h, st * 128:(st + 1) * 128],
                                qs[:, h, lsl],
                                start=True,
                                stop=True,
                                skip_group_check=True,
                            )
                        nc.scalar.activation(
                            out=expt[:, 2 * pr:2 * pr + 2, :],
                            in_=pscore,
                            func=mybir.ActivationFunctionType.Exp,
                            scale=SCALE,
                        )
                    # O-chunk filler from previous l-block keeps PE busy
                    if lb > 0 and h < 2:
                        emit_o_chunk(lb - 1, h)
                    # av: [d2, l] accumulated over s-tiles (reads expt first)
                    pav = ps_o.tile([128, 512], F32, tag="pav", name="pav")
                    for st in range(NST):
                        nc.tensor.matmul(
                            pav,
                            vs[:, st, h * D2:(h + 1) * D2],
                            expt[:, st, :],
                            start=(st == 0),
                            stop=(st == NST - 1),
                        )
                    if lb > 0 and h >= 2:
                        emit_o_chunk(lb - 1, h)
                    # in-place pairwise tree-sum of the 16 s-tiles
                    for step in (1, 2, 4, 8):
                        eng = nc.gpsimd if step == 1 else nc.vector
                        for j in range(0, NST, 2 * step):
                            eng.tensor_add(
                                out=expt[:, j, :], in0=expt[:, j, :], in1=expt[:, j + step, :]
                            )
                    pden = ps_d.tile([128, 512], F32, tag="pden", name="pden")
                    nc.tensor.matmul(pden, ones, expt[:, 0, :], start=True, stop=True)
                    recip = rc_pool.tile([128, 512], F32, tag="recip", name="recip")
                    nc.vector.reciprocal(out=recip, in_=pden)
                    hp, jj = h // 2, h % 2
                    r = slice(64 * jj, 64 * (jj + 1))
                    nc.vector.tensor_mul(out=ot3[r, 0, hp, :], in0=pav[0:64, :], in1=recip[0:64, :])
                    nc.vector.tensor_mul(
                        out=ot3[r, 1, hp, :], in0=pav[64:128, :], in1=recip[64:128, :]
                    )
                # sum stream for the O product m1
                nc.vector.tensor_add(
                    out=ot3[:, 2, :, :], in0=ot3[:, 0, :, :], in1=ot3[:, 1, :, :]
                )
            for half in range(4):
                emit_o_chunk(NLB - 1, half)


def _prep_shared(inputs):
    """Batch-shared transposed bf16 activation streams."""
    import ml_dtypes

    bf = ml_dtypes.bfloat16
    out = {}
    for b in range(B):
        out[b] = (
            np.ascontiguousarray(
                np.stack([inputs["inputs_real"][b].T, inputs["inputs_imag"][b].T]).astype(bf)
            ),
            np.ascontiguousarray(
                np.stack([inputs["context_real"][b].T, inputs["context_imag"][b].T]).astype(bf)
            ),
        )
    return out


def _prep_core_inputs(inputs, core, shared=None):
    """Slice + host-prepare the weight layouts for one core."""
    import ml_dtypes

    bf = ml_dtypes.bfloat16
    b = core // 4
    g = core % 4
    hcols = slice(g * HPC * D, (g + 1) * HPC * D)  # 256 channel cols/rows

    if shared is None:
        shared = _prep_shared(inputs)
    xt3, ct3 = shared[b]

    def stack_lhst(wr, wi):
        # [C, NHP, 3, 128]: streams (br, br+bi, bi-br), head-pair packed
        wrp = wr.reshape(C, NHP, D2)
        wip = wi.reshape(C, NHP, D2)
        return np.stack([wrp, wrp + wip, wip - wrp], axis=2).astype(bf)

    def stack_rhs_v(wr, wi):
        # [C, 3, HPC*D]
        return np.stack([wr, wr + wi, wi - wr], axis=1).astype(bf)

    def stack_wo(wr, wi):
        # [NHP, 128, 3, NEB, 512]: rows = head-pair packed d
        wrp = wr.reshape(NHP, D2, C)
        wip = wi.reshape(NHP, D2, C)
        st = np.stack([wrp, wrp + wip, wip - wrp], axis=2)  # [NHP, 128, 3, C]
        return np.ascontiguousarray(st.reshape(NHP, D2, 3, NEB, 512)).astype(bf)

    return {
        "xt": xt3,
        "ct": ct3,
        "wq": stack_lhst(inputs["wq_r"][:, hcols], inputs["wq_i"][:, hcols]),
        "wk": stack_lhst(inputs["wk_r"][:, hcols], inputs["wk_i"][:, hcols]),
        "wv": stack_rhs_v(inputs["wv_r"][:, hcols], inputs["wv_i"][:, hcols]),
        "wo": stack_wo(inputs["wo_r"][hcols, :], inputs["wo_i"][hcols, :]),
    }


def get_program():
    if "nc" not in _CACHE:
        _CACHE["nc"] = _build_program()
    return _CACHE["nc"]


def kernel(**inputs):
    nc = get_program()
    shared = _prep_shared(inputs)
    in_maps = [_prep_core_inputs(inputs, core, shared) for core in range(8)]
    res = run_bass_kernel_spmd(nc, in_maps, core_ids=list(range(8)))

    yr = np.zeros((B, L, C), np.float32)
    yi = np.zeros((B, L, C), np.float32)
    for core in range(8):
        b = core // 4
        yr[b] += res.results[core]["y_r"]
        yi[b] += res.results[core]["y_i"]
    yr += inputs["bo_r"][None, None, :]
    yi += inputs["bo_i"][None, None, :]
    return np.stack([yr, yi], axis=0)


# revision 24
# speedup vs baseline: 1.1831x; 1.0426x over previous
"""Trainium2 Bass kernel for nn_ComplexCrossAttention.

Sharding: 8 cores = 2 batches x 4 head-groups (4 heads each).

Structure (v3):
  - Activations arrive pre-transposed, pre-cast to bf16 on host:
    xt/ct = [2, C, L] = (re^T, im^T). The Karatsuba sum stream is
    formed on-chip (DVE, 2-byte fast mode).
  - All four complex projections (Q, K, V, O) use a 3-multiplication
    complex product with only TWO combine ops:
        m1 = (ar+ai) @ br ; m2 = ai @ (br+bi) ; m3 = ar @ (bi-br)
        re = m1 - m2 ; im = m1 + m3
    (weight sums/differences are precomputed on host for free).
    This cuts projection PE time 25% vs the 4-mult stacked form.
  - q/k in fp16 (same PE rate as bf16, 8x finer mantissa -> score
    logits keep baseline precision).
  - scoresT = (qr.kr + qi.ki)*scale with s on partitions; exp on Act;
    in-place pairwise tree-sum of the 16 s-tiles split Pool/DVE;
    cross-partition denominator via one ones-matmul; normalize on DVE.
  - O-projection in two PSUM waves (m1,m2 then m3, with m1 stashed to
    SBUF) so the attention inner loop fits exactly 8 PSUM banks;
    O-chunks of l-block lb-1 are interleaved between the heads of lb
    to keep PE fed while Act catches up on exp.
Host sums the 4 partial y per batch and adds the bias.
"""

import sys

import numpy as np

try:
    import concourse.bacc as bacc
except ImportError:  # pragma: no cover - fallback for bare environments
    sys.path.insert(0, "/opt/trn_rl_repo")
    import concourse.bacc as bacc

import concourse.mybir as mybir
import concourse.tile as tile
from concourse.bass_utils import run_bass_kernel_spmd

F32 = mybir.dt.float32
F16 = mybir.dt.float16
BF16 = mybir.dt.bfloat16

# ---- problem constants (hardcoded per contract) ----
B, L, S, C = 2, 2048, 2048, 1024
H, D = 16, 64
SCALE = float(1.0 / np.sqrt(np.float32(D)))
HPC = 4          # heads per core
NHP = 2          # head pairs per core
D2 = 2 * D       # stacked (re|im) head dim = 128
NCK = C // 128   # contraction chunks = 8
NLB = L // 512   # l-blocks = 4
NST = S // 128   # s-tiles = 16
NEB = 2          # e-blocks of 512 in C

QS_DT = F16      # scores operands
EXP_DT = BF16    # expT / vs / ones
OT_DT = BF16     # ot / wo

_CACHE = {}


def _build_program():
    nc = bacc.Bacc("TRN2", target_bir_lowering=False, debug=False, num_devices=8)

    xt = nc.dram_tensor("xt", [2, C, L], BF16, kind="ExternalInput")
    ct = nc.dram_tensor("ct", [2, C, S], BF16, kind="ExternalInput")
    # wq/wk: [C, NHP, 3, 128]; product streams t=0: br, 1: br+bi, 2: bi-br;
    # head pair packed along the last (M) axis.
    wq = nc.dram_tensor("wq", [C, NHP, 3, D2], BF16, kind="ExternalInput")
    wk = nc.dram_tensor("wk", [C, NHP, 3, D2], BF16, kind="ExternalInput")
    wv = nc.dram_tensor("wv", [C, 3, HPC * D], BF16, kind="ExternalInput")
    wo = nc.dram_tensor("wo", [NHP, D2, 3, NEB, 512], BF16, kind="ExternalInput")

    y_r = nc.dram_tensor("y_r", [L, C], F32, kind="ExternalOutput")
    y_i = nc.dram_tensor("y_i", [L, C], F32, kind="ExternalOutput")

    with tile.TileContext(nc) as tc:
        _emit(nc, tc, xt, ct, wq, wk, wv, wo, y_r, y_i)

    nc.compile()
    return nc


def _emit(nc, tc, xt, ct, wq, wk, wv, wo, y_r, y_i):
    from contextlib import ExitStack

    ctx = ExitStack()
    with ctx:
        attn_sb = ctx.enter_context(tc.tile_pool(name="attn_sb", bufs=1))

        qs = attn_sb.tile([128, HPC, L], QS_DT)            # [(qr|qi), h, l]
        ks = attn_sb.tile([128, HPC, S], QS_DT)            # [(kr|ki), h, s]
        vs = attn_sb.tile([128, NST, HPC * D2], EXP_DT)    # [s, st, (vr|vi) per head]

        proj_ctx = ExitStack()
        ct_pool = proj_ctx.enter_context(tc.tile_pool(name="ctp", bufs=1))
        ct_sb = ct_pool.tile([128, NCK, 2, S], BF16, name="ct_sb")

        # ---------- P1: Q projection ----------
        with (
            tc.tile_pool(name="xtp", bufs=1) as xt_pool,
            tc.tile_pool(name="wqp", bufs=1) as wq_pool,
            tc.tile_pool(name="xsp", bufs=2) as xs_pool,
            tc.tile_pool(name="psq", bufs=2, space="PSUM") as ps_q,
        ):
            wq_sb = wq_pool.tile([128, NCK, NHP, 3, D2], BF16, name="wq_sb")
            wq_r = wq.rearrange("(ck p) hp t m -> p ck hp t m", p=128)
            xt_sb = xt_pool.tile([128, NCK, 2, L], BF16, name="xt_sb")
            # interleave weight + activation loads so chunk ck is complete
            # (weights AND both streams) as early as possible
            for ck in range(NCK):
                csl = slice(ck * 128, (ck + 1) * 128)
                nc.sync.dma_start(out=wq_sb[:, ck], in_=wq_r[:, ck])
                for p in range(2):
                    nc.sync.dma_start(out=xt_sb[:, ck, p, :], in_=xt[p, csl, :])
            # context loads queue right behind (consumed by K/V)
            for ck in range(NCK):
                csl = slice(ck * 128, (ck + 1) * 128)
                for p in range(2):
                    nc.sync.dma_start(out=ct_sb[:, ck, p, :], in_=ct[p, csl, :])

            for lb in range(NLB):
                lsl = slice(lb * 512, (lb + 1) * 512)
                xstage = xs_pool.tile([128, NCK, 512], BF16, tag="xs", name="xstage")
                for ck in range(NCK):
                    nc.vector.tensor_add(
                        out=xstage[:, ck, :], in0=xt_sb[:, ck, 0, lsl], in1=xt_sb[:, ck, 1, lsl]
                    )
                for hp in range(NHP):
                    pt = ps_q.tile([128, 3, 512], F32, tag="pt", name="pt")
                    for ck in range(NCK):
                        rhs = (xstage[:, ck, :], xt_sb[:, ck, 1, lsl], xt_sb[:, ck, 0, lsl])
                        for p in range(3):
                            nc.tensor.matmul(
                                pt[:, p, :],
                                wq_sb[:, ck, hp, p, :],
                                rhs[p],
                                start=(ck == 0),
                                stop=(ck == NCK - 1),
                            )
                    for j in range(2):
                        h = 2 * hp + j
                        r = slice(64 * j, 64 * (j + 1))
                        nc.vector.tensor_sub(out=qs[0:64, h, lsl], in0=pt[r, 0, :], in1=pt[r, 1, :])
                        nc.gpsimd.tensor_add(
                            out=qs[64:128, h, lsl], in0=pt[r, 0, :], in1=pt[r, 2, :]
                        )

        # ---------- P2: K projection ----------
        wk_pool = proj_ctx.enter_context(tc.tile_pool(name="wkp", bufs=1))
        wk_sb = wk_pool.tile([128, NCK, NHP, 3, D2], BF16, name="wk_sb")
        wk_r = wk.rearrange("(ck p) hp t m -> p ck hp t m", p=128)
        for ck in range(NCK):
            nc.sync.dma_start(out=wk_sb[:, ck], in_=wk_r[:, ck])
        wv_sb = wk_pool.tile([128, NCK, 3, HPC * D], BF16, name="wv_sb")
        nc.sync.dma_start(out=wv_sb, in_=wv.rearrange("(ck p) t n -> p ck t n", p=128))
        cs_pool = proj_ctx.enter_context(tc.tile_pool(name="csp", bufs=1))
        cs_sb = cs_pool.tile([128, NCK, S], BF16, name="cs_sb")
        for ck in range(NCK):
            nc.vector.tensor_add(
                out=cs_sb[:, ck, :], in0=ct_sb[:, ck, 0, :], in1=ct_sb[:, ck, 1, :]
            )
        with tc.tile_pool(name="psk", bufs=2, space="PSUM") as ps_k:
            for hp in range(NHP):
                for sb in range(NLB):
                    ssl = slice(sb * 512, (sb + 1) * 512)
                    pt = ps_k.tile([128, 3, 512], F32, tag="pt", name="ptk")
                    for ck in range(NCK):
                        rhs = (cs_sb[:, ck, ssl], ct_sb[:, ck, 1, ssl], ct_sb[:, ck, 0, ssl])
                        for p in range(3):
                            nc.tensor.matmul(
                                pt[:, p, :],
                                wk_sb[:, ck, hp, p, :],
                                rhs[p],
                                start=(ck == 0),
                                stop=(ck == NCK - 1),
                            )
                    for j in range(2):
                        h = 2 * hp + j
                        r = slice(64 * j, 64 * (j + 1))
                        nc.vector.tensor_sub(out=ks[0:64, h, ssl], in0=pt[r, 0, :], in1=pt[r, 1, :])
                        nc.gpsimd.tensor_add(
                            out=ks[64:128, h, ssl], in0=pt[r, 0, :], in1=pt[r, 2, :]
                        )

        # ---------- P3: V projection ----------
        with tc.tile_pool(name="psv", bufs=2, space="PSUM") as ps_v:
            NV = HPC * D
            for st in range(NST):
                ssl = slice(st * 128, (st + 1) * 128)
                pv = ps_v.tile([128, 3, 512], F32, tag="pv", name="pv")
                for ck in range(NCK):
                    lhs = (cs_sb[:, ck, ssl], ct_sb[:, ck, 1, ssl], ct_sb[:, ck, 0, ssl])
                    for p in range(3):
                        nc.tensor.matmul(
                            pv[:, p, 0:NV],
                            lhs[p],
                            wv_sb[:, ck, p, :],
                            start=(ck == 0),
                            stop=(ck == NCK - 1),
                            skip_group_check=True,
                        )
                vw = vs[:, st, :].rearrange("p (h x) -> p h x", h=HPC)
                p0 = pv[:, 0, 0:NV].rearrange("p (h x) -> p h x", h=HPC)
                p1 = pv[:, 1, 0:NV].rearrange("p (h x) -> p h x", h=HPC)
                p2 = pv[:, 2, 0:NV].rearrange("p (h x) -> p h x", h=HPC)
                nc.vector.tensor_sub(out=vw[:, :, 0:64], in0=p0, in1=p1)
                nc.gpsimd.tensor_add(out=vw[:, :, 64:128], in0=p0, in1=p2)
        proj_ctx.close()

        # ---------- P4: attention + output projection ----------
        with (
            tc.tile_pool(name="late", bufs=1) as late_pool,
            tc.tile_pool(name="expp", bufs=3) as exp_pool,
            tc.tile_pool(name="otp", bufs=2) as ot_pool,
            tc.tile_pool(name="rcp", bufs=2) as rc_pool,
            tc.tile_pool(name="ysb", bufs=3) as ysb_pool,
        ):
            wo_sb = late_pool.tile([128, NHP, 3, NEB, 512], OT_DT, name="wo_sb")
            ones = late_pool.tile([128, D2], EXP_DT, name="ones")
            scr = late_pool.tile([128, 2], F32, name="scr")
            nc.vector.memset(ones, 1.0)
            nc.vector.memset(scr, 0.0)
            # dummy exp to pull the Act table load off the critical path
            nc.scalar.activation(out=scr, in_=scr, func=mybir.ActivationFunctionType.Exp)
            nc.sync.dma_start(out=wo_sb, in_=wo.rearrange("hp p t eb e -> p hp t eb e"))
            ot_tiles = {}

            def emit_o_chunk(lb, half):
                """Output projection for two (jt, eb) pairs of l-block lb."""
                ot3 = ot_tiles[lb]
                for jt_eb in range(2 * half, 2 * half + 2):
                    jt, eb = jt_eb // 2, jt_eb % 2
                    lrow = slice((lb * 4 + jt) * 128, (lb * 4 + jt + 1) * 128)
                    jsl = slice(jt * 128, (jt + 1) * 128)
                    esl = slice(eb * 512, (eb + 1) * 512)
                    # wave 1: m1 = os@wor, m2 = oi@(wor+woi)
                    ty = ps_y.tile([128, 2, 512], F32, tag="ty", name="ty")
                    for p, osrc in ((0, 2), (1, 1)):
                        for hp in range(NHP):
                            nc.tensor.matmul(
                                ty[:, p, :],
                                ot3[:, osrc, hp, jsl],
                                wo_sb[:, hp, p, eb, :],
                                start=(hp == 0),
                                stop=(hp == NHP - 1),
                            )
                    yr_t = ysb_pool.tile([128, 512], F32, tag="yr", name="yr_t")
                    nc.vector.tensor_sub(out=yr_t, in0=ty[:, 0, :], in1=ty[:, 1, :])
                    nc.sync.dma_start(out=y_r[lrow, esl], in_=yr_t)
                    u_t = ysb_pool.tile([128, 512], F32, tag="u", name="u_t")
                    nc.gpsimd.tensor_copy(out=u_t, in_=ty[:, 0, :])
                    # wave 2: m3 = or@(woi-wor) into the recycled bank pair
                    ty2 = ps_y.tile([128, 2, 512], F32, tag="ty", name="ty2")
                    for hp in range(NHP):
                        nc.tensor.matmul(
                            ty2[:, 0, :],
                            ot3[:, 0, hp, jsl],
                            wo_sb[:, hp, 2, eb, :],
                            start=(hp == 0),
                            stop=(hp == NHP - 1),
                        )
                    yi_t = ysb_pool.tile([128, 512], F32, tag="yi", name="yi_t")
                    nc.vector.tensor_add(out=yi_t, in0=ty2[:, 0, :], in1=u_t)
                    nc.sync.dma_start(out=y_i[lrow, esl], in_=yi_t)

            main = ExitStack()
            ps_s = main.enter_context(tc.tile_pool(name="pss", bufs=2, space="PSUM"))
            ps_o = main.enter_context(tc.tile_pool(name="pso", bufs=1, space="PSUM"))
            ps_d = main.enter_context(tc.tile_pool(name="psd", bufs=1, space="PSUM"))
            ps_y = main.enter_context(tc.tile_pool(name="psy", bufs=1, space="PSUM"))
            for lb in range(NLB):
                lsl = slice(lb * 512, (lb + 1) * 512)
                ot3 = ot_pool.tile([128, 3, NHP, 512], OT_DT, tag="ot", name="ot3")
                ot_tiles[lb] = ot3
                for h in range(HPC):
                    expt = exp_pool.tile([128, NST, 512], EXP_DT, tag="expt", name="expt")
                    for pr in range(NST // 2):
                        pscore = ps_s.tile([128, 2, 512], F32, tag="ps", name="pscore")
                        for j in range(2):
                            st = 2 * pr + j
                            nc.tensor.matmul(
                                pscore[:, j, :],
                                ks[:, h, st * 128:(st + 1) * 128],
                                qs[:, h, lsl],
                                start=True,
                                stop=True,
                                skip_group_check=True,
                            )
                        nc.scalar.activation(
                            out=expt[:, 2 * pr:2 * pr + 2, :],
                            in_=pscore,
                            func=mybir.ActivationFunctionType.Exp,
                            scale=SCALE,
                        )
                    # O-chunk filler from previous l-block keeps PE busy
                    if lb > 0 and h < 2:
                        emit_o_chunk(lb - 1, h)
                    # av: [d2, l] accumulated over s-tiles (reads expt first)
                    pav = ps_o.tile([128, 512], F32, tag="pav", name="pav")
                    for st in range(NST):
                        nc.tensor.matmul(
                            pav,
                            vs[:, st, h * D2:(h + 1) * D2],
                            expt[:, st, :],
                            start=(st == 0),
                            stop=(st == NST - 1),
                        )
                    if lb > 0 and h >= 2:
                        emit_o_chunk(lb - 1, h)
                    # in-place pairwise tree-sum of the 16 s-tiles
                    for step in (1, 2, 4, 8):
                        eng = nc.gpsimd if step == 1 else nc.vector
                        for j in range(0, NST, 2 * step):
                            eng.tensor_add(
                                out=expt[:, j, :], in0=expt[:, j, :], in1=expt[:, j + step, :]
                            )
                    pden = ps_d.tile([128, 512], F32, tag="pden", name="pden")
                    nc.tensor.matmul(pden, ones, expt[:, 0, :], start=True, stop=True)
                    recip = rc_pool.tile([128, 512], F32, tag="recip", name="recip")
                    nc.vector.reciprocal(out=recip, in_=pden)
                    hp, jj = h // 2, h % 2
                    r = slice(64 * jj, 64 * (jj + 1))
                    nc.vector.tensor_mul(out=ot3[r, 0, hp, :], in0=pav[0:64, :], in1=recip[0:64, :])
                    nc.vector.tensor_mul(
                        out=ot3[r, 1, hp, :], in0=pav[64:128, :], in1=recip[64:128, :]
                    )
                # sum stream for the O product m1
                nc.vector.tensor_add(
                    out=ot3[:, 2, :, :], in0=ot3[:, 0, :, :], in1=ot3[:, 1, :, :]
                )
            main.close()
            # tail: last l-block's output projection in single-wave form
            # (3 full banks x 2 bufs now that the attention pools are closed)
            with tc.tile_pool(name="psz", bufs=2, space="PSUM") as ps_z:
                ot3 = ot_tiles[NLB - 1]
                lb = NLB - 1
                for jt_eb in range(8):
                    jt, eb = jt_eb // 2, jt_eb % 2
                    lrow = slice((lb * 4 + jt) * 128, (lb * 4 + jt + 1) * 128)
                    jsl = slice(jt * 128, (jt + 1) * 128)
                    esl = slice(eb * 512, (eb + 1) * 512)
                    tz = ps_z.tile([128, 3, 512], F32, tag="tz", name="tz")
                    for p, osrc in ((0, 2), (1, 1), (2, 0)):
                        for hp in range(NHP):
                            nc.tensor.matmul(
                                tz[:, p, :],
                                ot3[:, osrc, hp, jsl],
                                wo_sb[:, hp, p, eb, :],
                                start=(hp == 0),
                                stop=(hp == NHP - 1),
                            )
                    yr_t = ysb_pool.tile([128, 512], F32, tag="yr", name="yr_tz")
                    nc.vector.tensor_sub(out=yr_t, in0=tz[:, 0, :], in1=tz[:, 1, :])
                    nc.sync.dma_start(out=y_r[lrow, esl], in_=yr_t)
                    yi_t = ysb_pool.tile([128, 512], F32, tag="yi", name="yi_tz")
                    nc.vector.tensor_add(out=yi_t, in0=tz[:, 0, :], in1=tz[:, 2, :])
                    nc.sync.dma_start(out=y_i[lrow, esl], in_=yi_t)


def _prep_shared(inputs):
    """Batch-shared transposed bf16 activation streams."""
    import ml_dtypes

    bf = ml_dtypes.bfloat16
    out = {}
    for b in range(B):
        out[b] = (
            np.ascontiguousarray(
                np.stack([inputs["inputs_real"][b].T, inputs["inputs_imag"][b].T]).astype(bf)
            ),
            np.ascontiguousarray(
                np.stack([inputs["context_real"][b].T, inputs["context_imag"][b].T]).astype(bf)
            ),
        )
    return out


def _prep_core_inputs(inputs, core, shared=None):
    """Slice + host-prepare the weight layouts for one core."""
    import ml_dtypes

    bf = ml_dtypes.bfloat16
    b = core // 4
    g = core % 4
    hcols = slice(g * HPC * D, (g + 1) * HPC * D)  # 256 channel cols/rows

    if shared is None:
        shared = _prep_shared(inputs)
    xt3, ct3 = shared[b]

    def stack_lhst(wr, wi):
        # [C, NHP, 3, 128]: streams (br, br+bi, bi-br), head-pair packed
        wrp = wr.reshape(C, NHP, D2)
        wip = wi.reshape(C, NHP, D2)
        return np.stack([wrp, wrp + wip, wip - wrp], axis=2).astype(bf)

    def stack_rhs_v(wr, wi):
        # [C, 3, HPC*D]
        return np.stack([wr, wr + wi, wi - wr], axis=1).astype(bf)

    def stack_wo(wr, wi):
        # [NHP, 128, 3, NEB, 512]: rows = head-pair packed d
        wrp = wr.reshape(NHP, D2, C)
        wip = wi.reshape(NHP, D2, C)
        st = np.stack([wrp, wrp + wip, wip - wrp], axis=2)  # [NHP, 128, 3, C]
        return np.ascontiguousarray(st.reshape(NHP, D2, 3, NEB, 512)).astype(bf)

    return {
        "xt": xt3,
        "ct": ct3,
        "wq": stack_lhst(inputs["wq_r"][:, hcols], inputs["wq_i"][:, hcols]),
        "wk": stack_lhst(inputs["wk_r"][:, hcols], inputs["wk_i"][:, hcols]),
        "wv": stack_rhs_v(inputs["wv_r"][:, hcols], inputs["wv_i"][:, hcols]),
        "wo": stack_wo(inputs["wo_r"][hcols, :], inputs["wo_i"][hcols, :]),
    }


def get_program():
    if "nc" not in _CACHE:
        _CACHE["nc"] = _build_program()
    return _CACHE["nc"]


def kernel(**inputs):
    nc = get_program()
    shared = _prep_shared(inputs)
    in_maps = [_prep_core_inputs(inputs, core, shared) for core in range(8)]
    res = run_bass_kernel_spmd(nc, in_maps, core_ids=list(range(8)))

    yr = np.zeros((B, L, C), np.float32)
    yi = np.zeros((B, L, C), np.float32)
    for core in range(8):
        b = core // 4
        yr[b] += res.results[core]["y_r"]
        yi[b] += res.results[core]["y_i"]
    yr += inputs["bo_r"][None, None, :]
    yi += inputs["bo_i"][None, None, :]
    return np.stack([yr, yi], axis=0)
